# revision 24
# baseline (speedup 1.0000x reference)
import numpy as np
import ml_dtypes

import concourse.bacc as bacc
import concourse.bass as bass
import concourse.mybir as mybir
import concourse.tile as tile
from concourse import bass_utils

bf16 = ml_dtypes.bfloat16

B, N, D = 4, 2048, 1024
NQ, NK = 1024, 2048
FP32 = mybir.dt.float32
BF16 = mybir.dt.bfloat16
FP16 = mybir.dt.float16
FP8 = mybir.dt.float8e4
EXP = mybir.ActivationFunctionType.Exp
SQRT = mybir.ActivationFunctionType.Sqrt

LAST_EXEC_NS = None
_NC = None


def _broadcast_ap(src_ap, parts):
    return bass.AP(
        tensor=src_ap.tensor,
        offset=src_ap.offset,
        ap=[[0, parts], src_ap.ap[-1]],
    )


def _two_part_ap(src_ap, stride, count):
    return bass.AP(
        tensor=src_ap.tensor,
        offset=src_ap.offset,
        ap=[[stride, count], src_ap.ap[-1]],
    )


def _build(debug=False):
    nc = bacc.Bacc(None, target_bir_lowering=False)
    qT = nc.dram_tensor("qT", [128, 8 * NQ], BF16, kind="ExternalInput")
    qn = nc.dram_tensor("qn", [NQ, D], FP32, kind="ExternalInput")
    kT = nc.dram_tensor("kT", [128, 8 * NK], BF16, kind="ExternalInput")
    vT = nc.dram_tensor("vT", [128, 8 * NK], BF16, kind="ExternalInput")
    wq = nc.dram_tensor("wq", [D, D], BF16, kind="ExternalInput")
    wk = nc.dram_tensor("wk", [D, D], BF16, kind="ExternalInput")
    wv = nc.dram_tensor("wv", [128, 8 * D], BF16, kind="ExternalInput")
    wo = nc.dram_tensor("wo", [128, 8 * D], BF16, kind="ExternalInput")
    gamma = nc.dram_tensor("gamma", [1, D], FP32, kind="ExternalInput")
    beta = nc.dram_tensor("beta", [1, D], FP32, kind="ExternalInput")
    out = nc.dram_tensor("out", [NQ, D], FP32, kind="ExternalOutput")
    dbg = nc.dram_tensor("dbg", [128, 4096], FP32, kind="ExternalOutput") if debug else None

    with tile.TileContext(nc) as tc:
        with (
            tc.tile_pool(name="perm", bufs=1) as perm,
            tc.tile_pool(name="ps", bufs=1, space="PSUM") as ps,
            tc.tile_pool(name="work", bufs=1) as work,
        ):
            gamma_t = perm.tile([128, D], FP32)
            beta_t = perm.tile([128, D], FP32)
            nc.gpsimd.dma_start(out=gamma_t, in_=_broadcast_ap(gamma[0:1, :], 128))
            nc.gpsimd.dma_start(out=beta_t, in_=_broadcast_ap(beta[0:1, :], 128))
            eps_t = perm.tile([128, 1], FP32)
            nc.vector.memset(eps_t, 1e-5)
            ones_sb = perm.tile([128, 1], FP16)
            nc.vector.memset(ones_sb, 1.0)

            qfull = perm.tile([128, 8, NQ], BF16, name="qfull")
            qstages = [qfull[:, :, 0:512], qfull[:, :, 512:1024]]

            ksT = [perm.tile([128, NK], FP8, name=f"ks{j}") for j in range(8)]
            vsp = [perm.tile([128, 1024], FP8, name=f"vsp{t}") for t in range(16)]
            qs = [[perm.tile([128, 512], BF16, name=f"qs{qc}_{j}")
                   for j in range(8)] for qc in range(2)]
            at_t = [[perm.tile([128, 512], BF16, name=f"at{qc}_{j}")
                     for j in range(8)] for qc in range(2)]

            # ---------- shared emission helpers ----------
            def emit_qproj_load(qc, jt, box):
                def fn():
                    wqp = work.tile([128, 8, 128], BF16, tag="wqp", bufs=3)
                    nc.sync.dma_start(
                        wqp,
                        wq[jt * 128:(jt + 1) * 128, :].rearrange(
                            "p (dt f) -> p dt f", dt=8))
                    box[0] = wqp
                return fn

            def emit_qproj_mms(qc, jt, box):
                def fn():
                    pp = ps.tile([128, 512], FP32, tag="pp", bufs=2)
                    for dt in range(8):
                        nc.tensor.matmul(
                            pp, box[0][:, dt, :], qstages[qc][:, dt, :],
                            start=(dt == 0), stop=(dt == 7))
                    nc.vector.tensor_copy(qs[qc][jt], pp)
                return fn

            def q_units(qc, jt):
                box = [None]
                return [emit_qproj_load(qc, jt, box),
                        emit_qproj_mms(qc, jt, box)]

            # ---------- normalization tail (emitted deferred) ----------
            def emit_norm(pend):
                qc, hp, uv, Eacc = pend
                denpA = ps.tile([128, 512], FP32, tag="pp", bufs=2)
                denpB = ps.tile([128, 512], FP32, tag="pp", bufs=2)
                nc.tensor.matmul(denpA[0:1, :], ones_sb, Eacc[:, 0:512],
                                 start=True, stop=True, skip_group_check=True)
                nc.tensor.matmul(denpB[0:1, :], ones_sb, Eacc[:, 512:1024],
                                 start=True, stop=True, skip_group_check=True)
                recips = work.tile([128, 1024], FP32, tag="recip", bufs=2)
                nc.vector.reciprocal_approx_fast(recips[0:1, 0:512],
                                                 denpA[0:1, :])
                nc.vector.reciprocal_approx_fast(recips[0:1, 512:1024],
                                                 denpB[0:1, :])
                rbA = work.tile([128, 512], FP32, tag="rb", bufs=4)
                rbB = work.tile([128, 512], FP32, tag="rb", bufs=4)
                nc.gpsimd.partition_broadcast(rbA, recips[0:1, 0:512])
                nc.gpsimd.partition_broadcast(rbB, recips[0:1, 512:1024])
                nc.vector.tensor_tensor(
                    at_t[qc][hp][0:64, :], uv[0:64, :], rbA[0:64, :],
                    mybir.AluOpType.mult)
                nc.vector.tensor_tensor(
                    at_t[qc][hp][64:128, :], uv[64:128, :], rbB[64:128, :],
                    mybir.AluOpType.mult)

            # ---------- attention block ----------
            def attention_block(qc, hp, extras, pending, inline_v=None):
                Es = {}

                def emit_sc(kt):
                    sc = ps.tile([128, 1024], FP32, tag="sc", bufs=2)
                    nc.tensor.matmul(
                        sc[:, 0:512],
                        ksT[hp][0:64, kt * 128:(kt + 1) * 128],
                        qs[qc][hp][0:64, :], start=True, stop=True,
                        skip_group_check=True)
                    nc.tensor.matmul(
                        sc[:, 512:1024],
                        ksT[hp][64:128, kt * 128:(kt + 1) * 128],
                        qs[qc][hp][64:128, :], start=True, stop=True,
                        tile_position=(64, 0), skip_group_check=True)
                    E = work.tile([128, 1024], FP16, tag="E", bufs=3)
                    nc.scalar.activation(E, sc, func=EXP, bias=0.0, scale=0.125)
                    Es[kt] = E

                uv = ps.tile([128, 512], FP32, tag="uv", bufs=2)
                Eacc = work.tile([128, 1024], FP16, tag="eacc", bufs=2)
                extras = list(extras)
                emit_sc(0)
                for kt in range(16):
                    if inline_v is not None and kt + 1 < 16:
                        inline_v(kt + 1)
                    if kt + 1 < 16:
                        emit_sc(kt + 1)
                    E = Es.pop(kt)
                    st = kt == 0
                    sp = kt == 15
                    nc.tensor.matmul(
                        uv[0:64, :],
                        vsp[kt][:, 128 * hp:128 * hp + 64],
                        E[:, 0:512], start=st, stop=sp,
                        skip_group_check=True)
                    nc.tensor.matmul(
                        uv[64:128, :],
                        vsp[kt][:, 128 * hp + 64:128 * hp + 128],
                        E[:, 512:1024], start=st, stop=sp,
                        skip_group_check=True)
                    if kt == 0:
                        nc.vector.tensor_copy(Eacc, E)
                    else:
                        nc.vector.tensor_add(out=Eacc, in0=Eacc, in1=E)
                    if kt == 6 and pending[0] is not None:
                        emit_norm(pending[0])
                        pending[0] = None
                    if extras and kt in (2, 5, 8, 11, 14):
                        extras.pop(0)()
                for fn in extras:
                    fn()
                return (qc, hp, uv, Eacc)

            # =========== phase 1: qc0 blocks + K/V/Q projections ===========
            with tc.tile_pool(name="kv", bufs=1) as kv:
                # critical-path small DMAs first on sync; spread bulk across
                # the three DMA-capable queues (sync / scalar / gpsimd)
                wv_t = kv.tile([128, 8, D], BF16, name="wv_t")
                kfull = kv.tile([128, 8, NK], BF16, name="kfull")
                wkp0 = kv.tile([128, 8, 128], BF16, tag="wkp", bufs=2)
                nc.sync.dma_start(
                    wkp0,
                    wk[0:128, :].rearrange("p (dt f) -> p dt f", dt=8))
                wqp00 = kv.tile([128, 8, 128], BF16, tag="wqp0", bufs=1)
                nc.sync.dma_start(
                    wqp00,
                    wq[0:128, :].rearrange("p (dt f) -> p dt f", dt=8))
                kview = kT[:, :].rearrange("p (dt n) -> p dt n", dt=8)
                nc.sync.dma_start(
                    qfull, qT[:, :].rearrange("p (dt n) -> p dt n", dt=8))
                nc.scalar.dma_start(kfull[:, 0:4, :], kview[:, 0:4, :])
                nc.scalar.dma_start(kfull[:, 4:8, :], kview[:, 4:8, :])
                nc.gpsimd.dma_start(
                    wv_t, wv[:, :].rearrange("p (dt f) -> p dt f", dt=8))

                vstage_all = {}
                vstage_cur = [None]

                vview = vT[:, :].rearrange("p (rc dt n) -> p rc dt n",
                                           rc=4, dt=8)

                def load_vstage(rc4):
                    vst = kv.tile([128, 8, 512], BF16, tag="vstage", bufs=4)
                    nc.gpsimd.dma_start(vst, vview[:, rc4, :, :])
                    vstage_cur[0] = vst
                    vstage_all[rc4] = vst

                def emit_vproj(rt, jc):
                    kt_dst = None  # computed by caller via closure below
                    pass

                def vproj_mms(kt, jc):
                    rc4, rt4 = divmod(kt, 4)
                    vst = vstage_all[rc4]
                    pp = ps.tile([128, 512], FP32, tag="pp", bufs=2)
                    for dt in range(8):
                        nc.tensor.matmul(
                            pp, vst[:, dt, rt4 * 128:(rt4 + 1) * 128],
                            wv_t[:, dt, jc * 512:(jc + 1) * 512],
                            start=(dt == 0), stop=(dt == 7))
                    nc.vector.tensor_copy(
                        vsp[kt][:, jc * 512:(jc + 1) * 512], pp)

                def inline_v(nkt):
                    rc4, rt4 = divmod(nkt, 4)
                    if rt4 == 0:
                        load_vstage(rc4)
                    vproj_mms(nkt, 0)

                def load_wkp(hp):
                    wkp = kv.tile([128, 8, 128], BF16, tag="wkp", bufs=2)
                    nc.sync.dma_start(
                        wkp,
                        wk[hp * 128:(hp + 1) * 128, :].rearrange(
                            "p (dt f) -> p dt f", dt=8))
                    return wkp

                def emit_kproj(hp, kc, wkp):
                    pp = ps.tile([128, 512], FP32, tag="pp", bufs=2)
                    for dt in range(8):
                        nc.tensor.matmul(
                            pp, wkp[:, dt, :],
                            kfull[:, dt, kc * 512:(kc + 1) * 512],
                            start=(dt == 0), stop=(dt == 7))
                    nc.vector.tensor_copy(
                        ksT[hp][:, kc * 512:(kc + 1) * 512], pp)

                def k_units(hp):
                    box = [None]

                    def loader():
                        box[0] = load_wkp(hp)
                    units = [loader]
                    for kc in range(4):
                        units.append(lambda kc=kc: emit_kproj(hp, kc, box[0]))
                    return units

                # prologue (weight DMAs already queued in header)
                pp0 = ps.tile([128, 512], FP32, tag="pp", bufs=2)
                for dt in range(8):
                    nc.tensor.matmul(
                        pp0, wqp00[:, dt, :], qstages[0][:, dt, :],
                        start=(dt == 0), stop=(dt == 7))
                nc.vector.tensor_copy(qs[0][0], pp0)
                load_vstage(0)
                vproj_mms(0, 0)
                for kc in range(4):
                    emit_kproj(0, kc, wkp0)
                vproj_mms(0, 1)

                extras_map = {}
                for hp in range(7):
                    extras_map[(0, hp)] = k_units(hp + 1) + q_units(0, hp + 1)
                vj1 = [(lambda kt=kt: vproj_mms(kt, 1)) for kt in range(16)]
                extras_map[(0, 1)] += vj1[0:5]
                extras_map[(0, 2)] += vj1[5:11]
                extras_map[(0, 3)] += vj1[11:16]
                extras_map[(0, 7)] = q_units(1, 0) + q_units(1, 1)

                pending = [None]
                for hp in range(8):
                    pend = attention_block(
                        0, hp, extras_map.get((0, hp), []), pending,
                        inline_v=inline_v if hp == 0 else None)
                    pending[0] = pend

            if dbg is not None:
                qsf = work.tile([128, 512], FP32, tag="rb", bufs=2)
                nc.vector.tensor_copy(qsf, qs[0][0])
                nc.sync.dma_start(dbg[:, 2560:3072], qsf)
                ksf = work.tile([128, 512], FP32, tag="rb", bufs=2)
                nc.vector.tensor_copy(ksf, ksT[0][:, 0:512])
                nc.sync.dma_start(dbg[:, 3072:3584], ksf)
                vsf = work.tile([128, 512], FP32, tag="rb", bufs=2)
                nc.vector.tensor_copy(vsf, vsp[0][:, 512:1024])
                nc.sync.dma_start(dbg[:, 3584:4096], vsf)
                atf = work.tile([128, 512], FP32, tag="rb", bufs=2)
                nc.vector.tensor_copy(atf, at_t[0][0])
                nc.sync.dma_start(dbg[:, 0:512], atf)

            # =========== phase 2: qc1 blocks + O proj / LN ===========
            with tc.tile_pool(name="tail", bufs=1) as tail:
                wo_t = [None]

                def load_wo():
                    wo_t[0] = tail.tile([128, 8, D], BF16, name="wo_t")
                    nc.gpsimd.dma_start(
                        wo_t[0],
                        wo[:, :].rearrange("p (dt f) -> p dt f", dt=8))

                outf_tiles = {}
                mv_tiles = {}

                def emit_oproj(qc, rt, oc):
                    row0 = qc * 512 + rt * 128
                    if oc == 0 and (qc, rt) not in outf_tiles:
                        outf = tail.tile([128, D], FP32, tag="outf", bufs=8)
                        nc.gpsimd.dma_start(outf, qn[row0:row0 + 128, :])
                        outf_tiles[(qc, rt)] = outf
                    outf = outf_tiles[(qc, rt)]
                    po = ps.tile([128, 512], FP32, tag="pp", bufs=2)
                    for it in range(8):
                        nc.tensor.matmul(
                            po, at_t[qc][it][:, rt * 128:(rt + 1) * 128],
                            wo_t[0][:, it, oc * 512:(oc + 1) * 512],
                            start=(it == 0), stop=(it == 7))
                    nc.vector.tensor_add(
                        out=outf[:, oc * 512:(oc + 1) * 512],
                        in0=outf[:, oc * 512:(oc + 1) * 512], in1=po)
                    if oc == 1:
                        bst = tail.tile([128, 2, 6], FP32, tag="bst", bufs=4)
                        mv = tail.tile([128, 2], FP32, tag="mv", bufs=8)
                        for sg in range(2):
                            nc.vector.bn_stats(
                                out=bst[:, sg, :],
                                in_=outf[:, sg * 512:(sg + 1) * 512])
                        nc.vector.bn_aggr(out=mv, in_=bst)
                        mv_tiles[(qc, rt)] = mv

                def emit_ln_rt(qc, rt):
                    row0 = qc * 512 + rt * 128
                    outf = outf_tiles[(qc, rt)]
                    mv = mv_tiles[(qc, rt)]
                    nc.scalar.activation(
                        out=mv[:, 1:2], in_=mv[:, 1:2], func=SQRT,
                        bias=eps_t[:, :], scale=1.0)
                    nc.vector.reciprocal(mv[:, 1:2], mv[:, 1:2])
                    y = tail.tile([128, D], FP32, tag="y", bufs=2)
                    nc.vector.tensor_scalar(
                        out=y, in0=outf, scalar1=mv[:, 0:1],
                        scalar2=mv[:, 1:2],
                        op0=mybir.AluOpType.subtract,
                        op1=mybir.AluOpType.mult)
                    nc.vector.tensor_mul(y, y, gamma_t)
                    nc.vector.tensor_add(out=y, in0=y, in1=beta_t)
                    nc.sync.dma_start(out[row0:row0 + 128, :], y)

                def emit_ln(qc):
                    for rt in range(4):
                        emit_ln_rt(qc, rt)

                def qn1_prefetch():
                    for rt in range(4):
                        outf = tail.tile([128, D], FP32, tag="outf", bufs=8)
                        nc.gpsimd.dma_start(
                            outf, qn[512 + rt * 128:512 + rt * 128 + 128, :])
                        outf_tiles[(1, rt)] = outf

                load_wo()
                for rt in range(4):
                    outf = tail.tile([128, D], FP32, tag="outf", bufs=8)
                    nc.gpsimd.dma_start(outf, qn[rt * 128:rt * 128 + 128, :])
                    outf_tiles[(0, rt)] = outf
                extras_map2 = {
                    (1, 0): q_units(1, 2),
                    (1, 1): q_units(1, 3) + [
                        lambda: emit_oproj(0, 0, 0), lambda: emit_oproj(0, 0, 1)],
                    (1, 2): q_units(1, 4) + [
                        lambda: emit_oproj(0, 1, 0), lambda: emit_oproj(0, 1, 1)],
                    (1, 3): q_units(1, 5) + [
                        lambda: emit_oproj(0, 2, 0), lambda: emit_oproj(0, 2, 1)],
                    (1, 4): q_units(1, 6) + [
                        lambda: emit_oproj(0, 3, 0), lambda: emit_oproj(0, 3, 1),
                        lambda: emit_ln(0)],
                    (1, 5): q_units(1, 7) + [qn1_prefetch],
                }
                for hp in range(8):
                    pend = attention_block(
                        1, hp, extras_map2.get((1, hp), []), pending)
                    pending[0] = pend
                emit_norm(pending[0])
                pending[0] = None

                for rt in range(4):
                    emit_oproj(1, rt, 0)
                    emit_oproj(1, rt, 1)
                    emit_ln_rt(1, rt)
    nc.finalize()
    return nc


def kernel(q, k, v, Wq, Wk, Wv, Wo, gamma, beta, _trace=False):
    global _NC, LAST_EXEC_NS
    if _NC is None:
        _NC = _build()
    def _pdtf(wT):
        # [dt*128+p, f] -> [p, dt, f] flattened to [128, 8*D]
        return np.ascontiguousarray(
            wT.reshape(8, 128, D).transpose(1, 0, 2).reshape(128, 8 * D)
        ).astype(bf16)

    def _jt_pdtf(wT):
        # [dt*128+p, jt*128+f] -> [jt*128+p, dt*128+f]
        return np.ascontiguousarray(
            wT.reshape(8, 128, 8, 128).transpose(2, 1, 0, 3).reshape(D, D)
        ).astype(bf16)

    wqh = _jt_pdtf(Wq.T.astype(np.float32))
    wkh = _jt_pdtf(Wk.T.astype(np.float32))
    wvh = _pdtf(Wv.T.astype(np.float32))
    woh = _pdtf(Wo.T.astype(np.float32))
    g = np.ascontiguousarray(np.asarray(gamma, dtype=np.float32).reshape(1, D))
    bt = np.ascontiguousarray(np.asarray(beta, dtype=np.float32).reshape(1, D))
    in_maps = []
    for c in range(8):
        b, hh = divmod(c, 2)
        qb = q[b, hh * NQ:(hh + 1) * NQ, :]
        in_maps.append({
            "qT": np.ascontiguousarray(
                qb.T.reshape(8, 128, NQ).transpose(1, 0, 2).reshape(
                    128, 8 * NQ)).astype(bf16),
            "qn": np.ascontiguousarray(qb, dtype=np.float32),
            "kT": np.ascontiguousarray(
                k[b].T.reshape(8, 128, NK).transpose(1, 0, 2).reshape(
                    128, 8 * NK)).astype(bf16),
            "vT": np.ascontiguousarray(
                v[b].T.reshape(8, 128, 4, 512).transpose(1, 2, 0, 3).reshape(
                    128, 8 * NK)).astype(bf16),
            "wq": wqh, "wk": wkh, "wv": wvh, "wo": woh,
            "gamma": g, "beta": bt,
        })
    res = bass_utils.run_bass_kernel_spmd(_NC, in_maps, list(range(8)), trace=_trace)
    LAST_EXEC_NS = getattr(res, "exec_time_ns", None)
    outp = np.empty((B, N, D), np.float32)
    for c in range(8):
        b, hh = divmod(c, 2)
        outp[b, hh * NQ:(hh + 1) * NQ, :] = res.results[c]["out"]
    return outp
